# revision 19
# baseline (speedup 1.0000x reference)
"""Trainium2 Bass kernel: pre-norm decoder block (B=2, S=2048, D=1024, H=16, DFF=4096).

Sharding: 8 cores = 2 data-parallel groups (one per batch) x 4 tensor-parallel
ranks. Within a group, attention is head-sharded (4 heads/core, Megatron-style
column-parallel QKV). The head-sharded attention output is redistributed with
two chunked AllToAlls (one per head-pair) so each rank ends up with the full
1024 attention features for its own 512 sequence positions; the rest of the
block (Wo + residual + LN2 + FFN + residual) runs sequence-sharded with the
full Wo/w1/w2, so no further collectives are needed. The AllToAll moves 4x
fewer bytes than the equivalent AllGather and each chunk overlaps with either
the second half of attention or the first Wo pass.

All matmul operands are bf16 (fp32 PSUM accumulation): halves HBM traffic,
SBUF footprint and collective payload, enables the PE fast-weight-load path
and DVE 2x/4x modes. On-chip activations live in transposed layout
([feature, seq]) so no on-chip transposes are needed. LayerNorm statistics
are ones-vector matmuls on the tensor engine (partition-axis reduction in
this layout), pipelined in two seq-halves; the per-position LN affine is
folded into the projections via extra contraction rows.
"""

import numpy as np
import ml_dtypes

import concourse.bass as bass
import concourse.mybir as mybir
import concourse.tile as tile
from concourse import bacc
from concourse.bass_utils import run_bass_kernel_spmd

BF = mybir.dt.bfloat16
FP8 = mybir.dt.float8e4
F32 = mybir.dt.float32
AF = mybir.ActivationFunctionType
ALU = mybir.AluOpType
NPBF = ml_dtypes.bfloat16

B, S, D, H, DFF = 2, 2048, 1024, 16, 4096
DH = D // H
EPS = 1e-5

N_CORES = 8
TP = 4                    # tensor-parallel ranks per group
HC = H // TP              # heads per core
DC = HC * DH              # head features per core
RC = S // TP              # seq rows per core in stage B
FT = D // 128             # feature tiles
SB = S // 512             # 512-wide seq blocks
MT = DFF // 128           # dff tiles
REPLICA_GROUPS = [[0, 1, 2, 3], [4, 5, 6, 7]]


def build(repeat=1, qkv_bias=False):
    nc = bacc.Bacc("TRN2", target_bir_lowering=False, debug=False,
                   num_devices=N_CORES)

    d = {"qkv_bias": qkv_bias}
    d["xt"] = nc.dram_tensor("xt", [128, FT, S], BF, kind="ExternalInput")
    d["xres"] = nc.dram_tensor("xres", [128, FT, RC], BF, kind="ExternalInput")
    d["wq"] = nc.dram_tensor("wq", [2, 128, FT, 128], BF, kind="ExternalInput")
    d["wk"] = nc.dram_tensor("wk", [2, 128, FT, 128], BF, kind="ExternalInput")
    d["wv"] = nc.dram_tensor("wv", [128, FT, DC], BF, kind="ExternalInput")
    d["qkvc"] = nc.dram_tensor("qkvc", [6, DC], BF, kind="ExternalInput")
    # wo indexed [p, j, g, dd, c]: contraction tile f = 2g + j
    d["wo"] = nc.dram_tensor("wo", [128, 2, TP, FT, 128], BF, kind="ExternalInput")
    d["w1"] = nc.dram_tensor("w1", [MT, 128, FT, 128], BF, kind="ExternalInput")
    d["b1t"] = nc.dram_tensor("b1t", [128, MT], F32, kind="ExternalInput")
    d["w2"] = nc.dram_tensor("w2", [FT, 128, MT, 128], BF, kind="ExternalInput")
    d["b2t"] = nc.dram_tensor("b2t", [128, FT], F32, kind="ExternalInput")
    d["mask4"] = nc.dram_tensor("mask4", [4, 128, 512], BF, kind="ExternalInput")
    d["recipd"] = nc.dram_tensor("recipd", [128, 1], BF, kind="ExternalInput")
    d["ones64"] = nc.dram_tensor("ones64", [128, 16, HC, 1], BF, kind="ExternalInput")
    d["colsel"] = nc.dram_tensor("colsel", [1, 1], mybir.dt.uint32,
                                 kind="ExternalInput")
    d["out"] = nc.dram_tensor("out", [128, FT, RC], F32, kind="ExternalOutput")

    with tile.TileContext(nc) as tc:
        for _ in range(repeat):
            _emit(nc, tc, d)

    nc.compile()
    return nc


def _emit(nc, tc, d):
    qkv_bias = d["qkv_bias"]
    with (
        tc.tile_pool(name="dram", bufs=1, space="DRAM") as dramp,
        tc.tile_pool(name="wbig", bufs=1) as wbig,
        tc.tile_pool(name="outer", bufs=1) as outp,
    ):
        # DRAM bounce buffers for the two AllGather chunks (one per head pair)
        ag_in = [dramp.tile([128, S], FP8, tag=f"ag_in{j}",
                            name=f"ag_in{j}") for j in range(2)]
        ag_out = [dramp.tile([TP * 128, S], FP8, tag=f"ag_out{j}",
                             name=f"ag_out{j}") for j in range(2)]
        recipd = outp.tile([128, 1], BF, tag="recipd")
        nc.sync.dma_start(out=recipd[:], in_=d["recipd"].ap())
        # FFN / Wo weights prefetched during attention; consumed in stage B.
        wo_sb = wbig.tile([128, 2, TP, FT, 128], BF, tag="wo")
        bias_sb = outp.tile([128, MT + FT], F32, tag="bias")
        xres_sb = outp.tile([128, FT, RC], BF, tag="xres")

        # ============================ stage A ============================
        with tc.tile_pool(name="persa", bufs=1) as pa:
            qt_sb = pa.tile([128, 2, S], BF, tag="qt")
            kt_sb = pa.tile([128, 2, S], BF, tag="kt")
            v_sb = pa.tile([128, 16, HC, 65], BF, tag="v")
            nc.sync.dma_start(out=v_sb[:, :, :, 64:65], in_=d["ones64"].ap())

            with tc.tile_pool(name="xpool", bufs=1) as xp:
                x_sb = xp.tile([128, FT, S], BF, tag="x")
                for quarter in range(4):
                    hl = slice(512 * quarter, 512 * quarter + 512)
                    for f in range(FT):
                        nc.sync.dma_start(out=x_sb[:, f, hl],
                                          in_=d["xt"].ap()[:, f, hl])
                wq_sb = xp.tile([128, 2, FT, 128], BF, tag="wq")
                wk_sb = xp.tile([128, 2, FT, 128], BF, tag="wk")
                wv_sb = xp.tile([128, FT, DC], BF, tag="wv")
                for dd in range(2):
                    nc.sync.dma_start(out=wq_sb[:, dd], in_=d["wq"].ap()[dd])
                    nc.sync.dma_start(out=wk_sb[:, dd], in_=d["wk"].ap()[dd])
                nc.sync.dma_start(out=wv_sb[:], in_=d["wv"].ap())
                ncst = 6 if qkv_bias else 3
                cst = [xp.tile([1, DC], BF, tag=f"qkvc{i}", name=f"qkvc{i}")
                       for i in range(ncst)]
                for i in range(ncst):
                    nc.sync.dma_start(out=cst[i][:], in_=d["qkvc"].ap()[i:i + 1, :])
                wqs, wks, wvs = cst[0][:], cst[1][:], cst[2][:]
                if qkv_bias:
                    bqc, bkc, bvc = cst[3][:], cst[4][:], cst[5][:]
                else:
                    bqc = bkc = bvc = None

                # per-seq-block LN1 rows
                rs_row = xp.tile([1, S], F32, tag="rs_row")
                nmu_row = xp.tile([1, S], BF, tag="nmu_row")
                std_row = xp.tile([1, S], BF, tag="std_row") if qkv_bias else None
                rsb_row = xp.tile([1, S], BF, tag="rsb_row")
                a_b = xp.tile([128, S], BF, tag="a_b")
                rst = xp.tile([128, S // 128], F32, tag="rst")

                # ---- LN1 statistics (two waves of 2 seq blocks) ------
                with (
                    tc.tile_pool(name="stps", bufs=4, space="PSUM") as stps,
                    tc.tile_pool(name="sq", bufs=3) as sqp,
                    tc.tile_pool(name="rows", bufs=2) as rowp,
                ):
                    st = [stps.tile([2, 512], F32, tag="stmu", name=f"stmu{_s}")
                          for _s in range(SB)]
                    stm2 = [stps.tile([2, 512], F32, tag="stm2", name=f"stm2{_s}")
                            for _s in range(SB)]
                    for wave in range(2):
                        for f in range(FT):
                            wsl = slice(1024 * wave, 1024 * wave + 1024)
                            x2 = sqp.tile([128, 1024], BF, tag="x2")
                            nc.vector.tensor_tensor(x2[:], x_sb[:, f, wsl],
                                                    x_sb[:, f, wsl], ALU.mult)
                            for si in range(2):
                                s = 2 * wave + si
                                sl = bass.ts(s, 512)
                                s2 = bass.ts(si, 512)
                                nc.tensor.matmul(st[s][0:1, :], recipd[:],
                                                 x_sb[:, f, sl],
                                                 start=(f == 0),
                                                 stop=(f == FT - 1))
                                nc.tensor.matmul(stm2[s][0:1, :], recipd[:],
                                                 x2[:, s2],
                                                 start=(f == 0),
                                                 stop=(f == FT - 1))
                        # postproc for this wave's two blocks
                        for si in range(2):
                            s = 2 * wave + si
                            sl = bass.ts(s, 512)
                            mu_r = rowp.tile([1, 512], F32, tag="mu_r")
                            m2_r = rowp.tile([1, 512], F32, tag="m2_r")
                            var_r = rowp.tile([1, 512], F32, tag="var_r")
                            lnv_r = rowp.tile([1, 512], F32, tag="lnv_r")
                            nc.scalar.copy(mu_r[:], st[s][0:1, :])
                            nc.scalar.copy(m2_r[:], stm2[s][0:1, :])
                            nc.vector.tensor_tensor(var_r[:], mu_r[:],
                                                    mu_r[:], ALU.mult)
                            nc.vector.scalar_tensor_tensor(
                                var_r[:], m2_r[:], EPS,
                                var_r[:], op0=ALU.add, op1=ALU.subtract)
                            nc.vector.tensor_scalar(
                                out=nmu_row[:, sl], in0=mu_r[:],
                                scalar1=-1.0, scalar2=None, op0=ALU.mult)
                            nc.scalar.activation(lnv_r[:], var_r[:], AF.Ln)
                            nc.scalar.activation(rs_row[:, sl], lnv_r[:],
                                                 AF.Exp, scale=-0.5)
                            nc.scalar.copy(rsb_row[:, sl], rs_row[:, sl])
                            if qkv_bias:
                                nc.scalar.activation(std_row[:, sl], lnv_r[:],
                                                     AF.Exp, scale=0.5)
                            nc.gpsimd.partition_broadcast(a_b[:, sl],
                                                          rsb_row[:, sl])
                # rs for v-scale: roundtrip to get [128, 16] per-block scalars
                drs = dramp.tile([1, S], F32, tag="drs")
                nc.sync.dma_start(out=drs[:], in_=rs_row[:])
                nc.sync.dma_start(
                    out=rst[:],
                    in_=drs[:].rearrange("o (t p) -> (o p) t", p=128))

                # ---- projections -----------------------------------
                with tc.tile_pool(name="prps", bufs=3, space="PSUM") as prps:
                    for (w_sb, wsum, bc, o_sb) in ((wq_sb, wqs, bqc, qt_sb),
                                                   (wk_sb, wks, bkc, kt_sb)):
                        for dd in range(2):
                            dsl = bass.ts(dd, 128)
                            for s in range(SB):
                                sl = bass.ts(s, 512)
                                ps = prps.tile([128, 512], F32, tag="pp")
                                for f in range(FT):
                                    nc.tensor.matmul(ps[:], w_sb[:, dd, f],
                                                     x_sb[:, f, sl],
                                                     start=(f == 0), stop=False)
                                nc.tensor.matmul(ps[:], wsum[0:1, dsl],
                                                 nmu_row[:, sl], start=False,
                                                 stop=not qkv_bias)
                                if qkv_bias:
                                    nc.tensor.matmul(ps[:], bc[0:1, dsl],
                                                     std_row[:, sl],
                                                     start=False, stop=True)
                                nc.vector.tensor_tensor(o_sb[:, dd, sl], ps[:],
                                                        a_b[:, sl], ALU.mult)
                    for i in range(16):
                        rl = bass.ts(i, 128)
                        ps = prps.tile([128, DC], F32, tag="pv")
                        for f in range(FT):
                            nc.tensor.matmul(ps[:], x_sb[:, f, rl], wv_sb[:, f],
                                             start=(f == 0), stop=False)
                        nc.tensor.matmul(ps[:], nmu_row[:, rl], wvs,
                                         start=False, stop=not qkv_bias)
                        if qkv_bias:
                            nc.tensor.matmul(ps[:], std_row[:, rl], bvc,
                                             start=False, stop=True)
                        nc.vector.tensor_scalar(
                            out=v_sb[:, i, :, 0:64],
                            in0=ps[:].rearrange("p (h e) -> p h e", h=HC),
                            scalar1=rst[:, i:i + 1], scalar2=None,
                            op0=ALU.mult)
            # x pool closed

            # ---- attention + chunked AllToAll ----------------------
            with (
                tc.tile_pool(name="attp", bufs=1) as atp_a,
                tc.tile_pool(name="scps", bufs=2, space="PSUM") as scps,
                tc.tile_pool(name="pvps", bufs=4, space="PSUM") as pvps,
                tc.tile_pool(name="exps", bufs=6) as expp,
                tc.tile_pool(name="rcps", bufs=3) as rcpp,
            ):
                attnt_sb = atp_a.tile([128, 2, S], FP8, tag="attnt")
                mask4 = atp_a.tile([128, 4, 512], BF, tag="mask4")
                nc.sync.dma_start(out=mask4[:],
                                  in_=d["mask4"].ap().rearrange("j p c -> p j c"))
                # prefetch stage-B weights while attention computes
                # (w1 allocated here so it reuses the freed x-pool space)
                w1_sb = wbig.tile([128, MT, FT, 128], BF, tag="w1")
                nc.sync.dma_start(out=wo_sb[:], in_=d["wo"].ap())
                for m in range(MT):
                    nc.sync.dma_start(out=w1_sb[:, m], in_=d["w1"].ap()[m])
                nc.sync.dma_start(out=bias_sb[:, 0:MT], in_=d["b1t"].ap())
                nc.sync.dma_start(out=bias_sb[:, MT:MT + FT], in_=d["b2t"].ap())
                nc.sync.dma_start(out=xres_sb[:], in_=d["xres"].ap())
                for hp in range(2):
                    heads = (2 * hp, 2 * hp + 1)
                    for qi in range(SB):
                        qsl = bass.ts(qi, 512)
                        nki = 4 * qi + 4
                        pv = {h: pvps.tile([65, 512], F32, tag="pv",
                                           name=f"pv{h}_{qi}") for h in heads}
                        for ki in range(nki):
                            # both heads' score matmuls run concurrently in
                            # distinct 64-row PE strips (row tiling)
                            sc = scps.tile([128, 2, 512], F32, tag="sc")
                            for u in range(2):
                                h = heads[u]
                                hs = slice(64 * u, 64 * u + 64)
                                nc.tensor.matmul(
                                    sc[:, u],
                                    kt_sb[hs, h // 2, bass.ts(ki, 128)],
                                    qt_sb[hs, h // 2, qsl],
                                    start=True, stop=True)
                            ex = expp.tile([128, 2, 512], BF, tag="ex")
                            nc.scalar.activation(
                                ex[:].rearrange("p u s -> p (u s)"),
                                sc[:].rearrange("p u s -> p (u s)"),
                                AF.Exp)
                            rel = 128 * ki - 512 * qi
                            if rel >= 0:
                                mw = rel + 128
                                for u in range(2):
                                    nc.vector.tensor_tensor(
                                        ex[:, u, 0:mw], ex[:, u, 0:mw],
                                        mask4[:, rel // 128, 0:mw],
                                        ALU.mult)
                            for u in range(2):
                                h = heads[u]
                                nc.tensor.matmul(pv[h][:],
                                                 v_sb[:, ki, h, :],
                                                 ex[:, u],
                                                 start=(ki == 0),
                                                 stop=(ki == nki - 1))
                        for h in heads:
                            hb = 64 * (h % 2)
                            rcp = rcpp.tile([1, 512], F32, tag="rcp")
                            rcpb = rcpp.tile([64, 512], F32, tag="rcpb")
                            nc.vector.reciprocal(rcp[:], pv[h][64:65, :])
                            nc.gpsimd.partition_broadcast(rcpb[:], rcp[:])
                            nc.vector.tensor_tensor(
                                attnt_sb[hb:hb + 64, hp, qsl],
                                pv[h][0:64, :], rcpb[:], ALU.mult)
                    # this head-pair is done for all S: ship its AllGather
                    nc.sync.dma_start(out=ag_in[hp][:],
                                      in_=attnt_sb[:, hp, :])
                    nc.gpsimd.collective_compute(
                        "AllGather", ALU.bypass, replica_groups=REPLICA_GROUPS,
                        ins=[ag_in[hp].opt()], outs=[ag_out[hp].opt()])
        # stage-A pools closed

        # ============================ stage B ============================
        creg = nc.alloc_registers(f"colsel_r_{nc.next_id()}")
        nc.regs_load(creg, d["colsel"].ap()[0:1, 0:1])
        colsv = nc.snap(creg, donate=True, min_val=0, max_val=S - RC)

        with tc.tile_pool(name="persb", bufs=1) as pb:
            h_sb = pb.tile([128, FT, RC], BF, tag="h")
            st2 = pb.tile([1, 4 * RC], F32, tag="st2")
            MU2, VAR2, LNV2 = 0, 2 * RC, 3 * RC
            rs2_row = pb.tile([1, RC], F32, tag="rs2_row")
            l2a = pb.tile([1, RC], BF, tag="l2a")
            l2b = pb.tile([1, RC], BF, tag="l2b")
            l2a_b = pb.tile([128, RC], BF, tag="l2a_b")
            l2b_b = pb.tile([128, RC], BF, tag="l2b_b")

            # ---- Wo (two passes over AllGather chunks) + residual ----
            with (
                tc.tile_pool(name="atin", bufs=1) as atp,
                tc.tile_pool(name="h0p", bufs=1) as h0p,
                tc.tile_pool(name="wops", bufs=3, space="PSUM") as wops,
                tc.tile_pool(name="st2ps", bufs=1, space="PSUM") as st2ps,
                tc.tile_pool(name="sq2", bufs=2) as sq2p,
            ):
                stp = st2ps.tile([2, RC], F32, tag="st2p")
                stp2 = st2ps.tile([2, RC], F32, tag="st2p2")
                at_f8 = atp.tile([128, 2, TP, RC], FP8, tag="at_f8")
                at_in = atp.tile([128, 2, TP, RC], BF, tag="at_in")
                h0_sb = h0p.tile([128, FT, RC], F32, tag="h0")
                for j in range(2):
                    bo_view = ag_out[j][:].rearrange("(g p) s -> p g s", p=128)
                    for g in range(TP):
                        nc.sync.dma_start(
                            out=at_f8[:, j, g, :],
                            in_=bo_view[:, g, bass.ds(colsv, RC)])
                        nc.vector.tensor_scalar(
                            out=at_in[:, j, g, :], in0=at_f8[:, j, g, :],
                            scalar1=1.0, scalar2=None, op0=ALU.mult)
                    for dd in range(FT):
                        ps = wops.tile([128, RC], F32, tag="wops")
                        for g in range(TP):
                            nc.tensor.matmul(ps[:], wo_sb[:, j, g, dd],
                                             at_in[:, j, g, :],
                                             start=(g == 0), stop=(g == TP - 1))
                        if j == 0:
                            nc.vector.tensor_tensor(h0_sb[:, dd, :], ps[:],
                                                    xres_sb[:, dd, :], ALU.add)
                        else:
                            nc.vector.tensor_tensor(h_sb[:, dd, :], ps[:],
                                                    h0_sb[:, dd, :], ALU.add)
                            h2 = sq2p.tile([128, RC], BF, tag="h2")
                            nc.vector.tensor_tensor(h2[:], h_sb[:, dd],
                                                    h_sb[:, dd], ALU.mult)
                            nc.tensor.matmul(stp[0:1, :], recipd[:],
                                             h_sb[:, dd, :],
                                             start=(dd == 0),
                                             stop=(dd == FT - 1))
                            nc.tensor.matmul(stp2[0:1, :], recipd[:], h2[:],
                                             start=(dd == 0),
                                             stop=(dd == FT - 1))
                nc.scalar.copy(st2[:, MU2:MU2 + RC], stp[0:1, :])
                nc.scalar.copy(st2[:, MU2 + RC:MU2 + 2 * RC], stp2[0:1, :])

            # ---- LN2 + FFN -----------------------------------------
            with (
                tc.tile_pool(name="hnp", bufs=1) as hnp,
                tc.tile_pool(name="ap_", bufs=1) as ap_,
                tc.tile_pool(name="w2s", bufs=3) as w2p,
                tc.tile_pool(name="outs", bufs=2) as outsp,
                tc.tile_pool(name="f1ps", bufs=3, space="PSUM") as f1ps,
                tc.tile_pool(name="f2ps", bufs=2, space="PSUM") as f2ps,
            ):
                nc.vector.tensor_tensor(st2[:, VAR2:VAR2 + RC],
                                        st2[:, MU2:MU2 + RC],
                                        st2[:, MU2:MU2 + RC], ALU.mult)
                nc.vector.scalar_tensor_tensor(st2[:, VAR2:VAR2 + RC],
                                               st2[:, MU2 + RC:MU2 + 2 * RC],
                                               EPS,
                                               st2[:, VAR2:VAR2 + RC],
                                               op0=ALU.add, op1=ALU.subtract)
                nc.scalar.activation(st2[:, LNV2:LNV2 + RC],
                                     st2[:, VAR2:VAR2 + RC], AF.Ln)
                nc.scalar.activation(rs2_row[:], st2[:, LNV2:LNV2 + RC],
                                     AF.Exp, scale=-0.5)
                nc.scalar.copy(l2a[:], rs2_row[:])
                nc.vector.scalar_tensor_tensor(l2b[:], st2[:, MU2:MU2 + RC],
                                               -1.0, rs2_row[:],
                                               op0=ALU.mult, op1=ALU.mult)
                nc.gpsimd.partition_broadcast(l2a_b[:], l2a[:])
                nc.gpsimd.partition_broadcast(l2b_b[:], l2b[:])

                hn_sb = hnp.tile([128, FT, RC], BF, tag="hn")
                for f in range(FT):
                    nc.vector.tensor_tensor(hn_sb[:, f, :], h_sb[:, f, :],
                                            l2a_b[:], ALU.mult)
                    nc.vector.tensor_add(hn_sb[:, f, :], hn_sb[:, f, :],
                                         l2b_b[:])

                a_sb = ap_.tile([128, MT, RC], BF, tag="a")
                for m in range(MT):
                    ps = f1ps.tile([128, RC], F32, tag="f1")
                    for f in range(FT):
                        nc.tensor.matmul(ps[:], w1_sb[:, m, f, :],
                                         hn_sb[:, f, :],
                                         start=(f == 0), stop=(f == FT - 1))
                    nc.scalar.activation(a_sb[:, m, :], ps[:], AF.Relu,
                                         bias=bias_sb[:, m:m + 1])

                for dd in range(FT):
                    w2d = w2p.tile([128, MT, 128], BF, tag="w2d")
                    nc.sync.dma_start(out=w2d[:], in_=d["w2"].ap()[dd])
                    ps = f2ps.tile([128, RC], F32, tag="f2")
                    for t in range(MT):
                        nc.tensor.matmul(ps[:], w2d[:, t, :], a_sb[:, t, :],
                                         start=(t == 0), stop=(t == MT - 1))
                    o_t = outsp.tile([128, RC], F32, tag="ot")
                    nc.vector.scalar_tensor_tensor(
                        o_t[:], ps[:],
                        bias_sb[:, MT + dd:MT + dd + 1],
                        h_sb[:, dd, :], op0=ALU.add, op1=ALU.add)
                    nc.sync.dma_start(out=d["out"].ap()[:, dd], in_=o_t[:])


# ----------------------------------------------------------------------
# host side
# ----------------------------------------------------------------------

def make_in_maps(x, mask, Wq, Wk, Wv, Wo, w1, b1, w2, b2, g1, be1, g2, be2):
    """Build the 8 per-core input maps from the full inputs."""
    f32 = np.float32
    x = np.asarray(x, f32)
    mask = np.asarray(mask)
    Wq, Wk, Wv, Wo = (np.asarray(t, f32) for t in (Wq, Wk, Wv, Wo))
    w1, b1, w2, b2 = (np.asarray(t, f32) for t in (w1, b1, w2, b2))
    g1, be1, g2, be2 = (np.asarray(t, f32) for t in (g1, be1, g2, be2))

    Wq_s = g1[:, None] * Wq / np.sqrt(np.float32(DH))
    Wk_s = g1[:, None] * Wk
    Wv_s = g1[:, None] * Wv
    bq_full = (be1 @ Wq) / np.sqrt(np.float32(DH))
    bk_full = be1 @ Wk
    bv_full = be1 @ Wv
    w1_s = g2[:, None] * w1
    b1_s = b1 + be2 @ w1
    m2d = np.asarray(mask[0, 0], bool)
    mask4 = np.stack([m2d[0:512, 128 * j:128 * j + 128].T.astype(f32)
                      for j in range(4)]).astype(NPBF)
    recipd = np.full((128, 1), 1.0 / D, NPBF)
    ones64 = np.ones((128, 16, HC, 1), NPBF)
    b1t = np.ascontiguousarray(b1_s.reshape(MT, 128).T)
    b2t = np.ascontiguousarray(b2.reshape(FT, 128).T)
    # wo[p, j, g, dd, c] = Wo[(2g+j)*128+p, dd*128+c]
    wo_p = np.ascontiguousarray(
        Wo.reshape(TP, 2, 128, FT, 128).transpose(2, 1, 0, 3, 4)).astype(NPBF)
    w1_p = np.ascontiguousarray(
        w1_s.reshape(FT, 128, MT, 128).transpose(2, 1, 0, 3)).astype(NPBF)
    w2_p = np.ascontiguousarray(
        w2.reshape(MT, 128, FT, 128).transpose(2, 1, 0, 3)).astype(NPBF)

    in_maps = []
    for c in range(N_CORES):
        g, r = divmod(c, TP)
        xT = np.ascontiguousarray(x[g].T)                       # [D, S]
        xt = np.ascontiguousarray(
            xT.reshape(FT, 128, S).transpose(1, 0, 2)).astype(NPBF)
        xres = np.ascontiguousarray(
            xT[:, RC * r:RC * (r + 1)].reshape(FT, 128, RC)
            .transpose(1, 0, 2)).astype(NPBF)
        sh = slice(DC * r, DC * (r + 1))
        wq_c = np.ascontiguousarray(
            Wq_s[:, sh].reshape(FT, 128, 2, 128).transpose(2, 1, 0, 3)
        ).astype(NPBF)
        wk_c = np.ascontiguousarray(
            Wk_s[:, sh].reshape(FT, 128, 2, 128).transpose(2, 1, 0, 3)
        ).astype(NPBF)
        wv_c = np.ascontiguousarray(
            Wv_s[:, sh].reshape(FT, 128, DC).transpose(1, 0, 2)).astype(NPBF)
        qkvc = np.stack([Wq_s[:, sh].sum(0), Wk_s[:, sh].sum(0),
                         Wv_s[:, sh].sum(0), bq_full[sh], bk_full[sh],
                         bv_full[sh]]).astype(NPBF)
        in_maps.append({
            "xt": xt, "xres": xres, "wq": wq_c, "wk": wk_c, "wv": wv_c,
            "qkvc": qkvc, "wo": wo_p, "w1": w1_p, "b1t": b1t, "w2": w2_p,
            "b2t": b2t, "mask4": mask4, "recipd": recipd,
            "ones64": ones64,
            "colsel": np.array([[RC * r]], np.uint32),
        })
    return in_maps


def assemble_output(results):
    """[8 x {out: [128, FT, RC]}] -> [B, S, D] float32."""
    out = np.empty((B, S, D), np.float32)
    for c in range(N_CORES):
        g, r = divmod(c, TP)
        ot = results[c]["out"].transpose(1, 0, 2).reshape(D, RC)  # [D, RC]
        out[g, RC * r:RC * (r + 1), :] = ot.T
    return out


_nc_cache = {}


def get_nc(repeat=1, qkv_bias=False, **_ignored):
    key = (repeat, qkv_bias)
    if key not in _nc_cache:
        _nc_cache[key] = build(repeat=repeat, qkv_bias=qkv_bias)
    return _nc_cache[key]


def kernel(**inputs):
    qkv_bias = bool(np.any(np.asarray(inputs["be1"], np.float32)))
    nc = get_nc(qkv_bias=qkv_bias)
    in_maps = make_in_maps(**inputs)
    res = run_bass_kernel_spmd(nc, in_maps, core_ids=list(range(N_CORES)))
    return assemble_output(res.results)


# revision 20
# speedup vs baseline: 1.0256x; 1.0256x over previous
"""Trainium2 Bass kernel: pre-norm decoder block (B=2, S=2048, D=1024, H=16, DFF=4096).

Sharding: 8 cores = 2 data-parallel groups (one per batch) x 4 tensor-parallel
ranks. Within a group, attention is head-sharded (4 heads/core, Megatron-style
column-parallel QKV). The head-sharded attention output is redistributed with
two chunked AllToAlls (one per head-pair) so each rank ends up with the full
1024 attention features for its own 512 sequence positions; the rest of the
block (Wo + residual + LN2 + FFN + residual) runs sequence-sharded with the
full Wo/w1/w2, so no further collectives are needed. The AllToAll moves 4x
fewer bytes than the equivalent AllGather and each chunk overlaps with either
the second half of attention or the first Wo pass.

All matmul operands are bf16 (fp32 PSUM accumulation): halves HBM traffic,
SBUF footprint and collective payload, enables the PE fast-weight-load path
and DVE 2x/4x modes. On-chip activations live in transposed layout
([feature, seq]) so no on-chip transposes are needed. LayerNorm statistics
are ones-vector matmuls on the tensor engine (partition-axis reduction in
this layout), pipelined in two seq-halves; the per-position LN affine is
folded into the projections via extra contraction rows.
"""

import numpy as np
import ml_dtypes

import concourse.bass as bass
import concourse.mybir as mybir
import concourse.tile as tile
from concourse import bacc
from concourse.bass_utils import run_bass_kernel_spmd

BF = mybir.dt.bfloat16
FP8 = mybir.dt.float8e4
F32 = mybir.dt.float32
AF = mybir.ActivationFunctionType
ALU = mybir.AluOpType
NPBF = ml_dtypes.bfloat16

B, S, D, H, DFF = 2, 2048, 1024, 16, 4096
DH = D // H
EPS = 1e-5

N_CORES = 8
TP = 4                    # tensor-parallel ranks per group
HC = H // TP              # heads per core
DC = HC * DH              # head features per core
RC = S // TP              # seq rows per core in stage B
FT = D // 128             # feature tiles
SB = S // 512             # 512-wide seq blocks
MT = DFF // 128           # dff tiles
REPLICA_GROUPS = [[0, 1, 2, 3], [4, 5, 6, 7]]


def build(repeat=1, qkv_bias=False):
    nc = bacc.Bacc("TRN2", target_bir_lowering=False, debug=False,
                   num_devices=N_CORES)

    d = {"qkv_bias": qkv_bias}
    d["xt"] = nc.dram_tensor("xt", [128, FT, S], BF, kind="ExternalInput")
    d["xres"] = nc.dram_tensor("xres", [128, FT, RC], BF, kind="ExternalInput")
    d["wq"] = nc.dram_tensor("wq", [2, 128, FT, 128], BF, kind="ExternalInput")
    d["wk"] = nc.dram_tensor("wk", [2, 128, FT, 128], BF, kind="ExternalInput")
    d["wv"] = nc.dram_tensor("wv", [128, FT, DC], BF, kind="ExternalInput")
    d["qkvc"] = nc.dram_tensor("qkvc", [6, DC], BF, kind="ExternalInput")
    # wo indexed [p, j, g, dd, c]: contraction tile f = 2g + j
    d["wo"] = nc.dram_tensor("wo", [128, 2, TP, FT, 128], BF, kind="ExternalInput")
    d["w1"] = nc.dram_tensor("w1", [MT, 128, FT, 128], BF, kind="ExternalInput")
    d["b1t"] = nc.dram_tensor("b1t", [128, MT], F32, kind="ExternalInput")
    d["w2"] = nc.dram_tensor("w2", [FT, 128, MT, 128], BF, kind="ExternalInput")
    d["b2t"] = nc.dram_tensor("b2t", [128, FT], F32, kind="ExternalInput")
    d["mask4"] = nc.dram_tensor("mask4", [4, 128, 512], BF, kind="ExternalInput")
    d["recipd"] = nc.dram_tensor("recipd", [128, 1], BF, kind="ExternalInput")
    d["ones64"] = nc.dram_tensor("ones64", [128, 16, HC, 1], BF, kind="ExternalInput")
    d["colsel"] = nc.dram_tensor("colsel", [1, 1], mybir.dt.uint32,
                                 kind="ExternalInput")
    d["out"] = nc.dram_tensor("out", [128, FT, RC], F32, kind="ExternalOutput")

    with tile.TileContext(nc) as tc:
        for _ in range(repeat):
            _emit(nc, tc, d)

    nc.compile()
    return nc


def _emit(nc, tc, d):
    qkv_bias = d["qkv_bias"]
    with (
        tc.tile_pool(name="dram", bufs=1, space="DRAM") as dramp,
        tc.tile_pool(name="wbig", bufs=1) as wbig,
        tc.tile_pool(name="outer", bufs=1) as outp,
    ):
        # DRAM bounce buffers for the two AllGather chunks (one per head pair)
        ag_in = [dramp.tile([128, S], FP8, tag=f"ag_in{j}",
                            name=f"ag_in{j}") for j in range(2)]
        ag_out = [dramp.tile([TP * 128, S], FP8, tag=f"ag_out{j}",
                             name=f"ag_out{j}") for j in range(2)]
        recipd = outp.tile([128, 1], BF, tag="recipd")
        nc.sync.dma_start(out=recipd[:], in_=d["recipd"].ap())
        # FFN / Wo weights prefetched during attention; consumed in stage B.
        wo_sb = wbig.tile([128, 2, TP, FT, 128], BF, tag="wo")
        bias_sb = outp.tile([128, MT + FT], F32, tag="bias")
        xres_sb = outp.tile([128, FT, RC], BF, tag="xres")

        # ============================ stage A ============================
        with tc.tile_pool(name="persa", bufs=1) as pa:
            qt_sb = pa.tile([128, 2, S], BF, tag="qt")
            kt_sb = pa.tile([128, 2, S], BF, tag="kt")
            v_sb = pa.tile([128, 16, HC, 65], BF, tag="v")
            nc.sync.dma_start(out=v_sb[:, :, :, 64:65], in_=d["ones64"].ap())

            with tc.tile_pool(name="xpool", bufs=1) as xp:
                x_sb = xp.tile([128, FT, S], BF, tag="x")
                wq_sb = xp.tile([128, 2, FT, 128], BF, tag="wq")
                wk_sb = xp.tile([128, 2, FT, 128], BF, tag="wk")
                wv_sb = xp.tile([128, FT, DC], BF, tag="wv")
                nc.sync.dma_start(out=wq_sb[:, 0], in_=d["wq"].ap()[0])
                for quarter in range(4):
                    hl = slice(512 * quarter, 512 * quarter + 512)
                    for f in range(FT):
                        nc.sync.dma_start(out=x_sb[:, f, hl],
                                          in_=d["xt"].ap()[:, f, hl])
                nc.sync.dma_start(out=wq_sb[:, 1], in_=d["wq"].ap()[1])
                for dd in range(2):
                    nc.sync.dma_start(out=wk_sb[:, dd], in_=d["wk"].ap()[dd])
                nc.sync.dma_start(out=wv_sb[:], in_=d["wv"].ap())
                # HAM warmup: dummy matmuls over the resident wq tile keep
                # the PE busy (and unthrottled) while x streams from HBM
                with tc.tile_pool(name="warm", bufs=1, space="PSUM") as wmp:
                    warm = wmp.tile([2, 512], F32, tag="warm")
                    wqv = wq_sb[:, 0].rearrange("p f c -> p (f c)")
                    for i in range(20):
                        nc.tensor.matmul(warm[0:1, :],
                                         recipd[:], wqv[:, 0:512],
                                         start=True, stop=True)
                ncst = 6 if qkv_bias else 3
                cst = [xp.tile([1, DC], BF, tag=f"qkvc{i}", name=f"qkvc{i}")
                       for i in range(ncst)]
                for i in range(ncst):
                    nc.sync.dma_start(out=cst[i][:], in_=d["qkvc"].ap()[i:i + 1, :])
                wqs, wks, wvs = cst[0][:], cst[1][:], cst[2][:]
                if qkv_bias:
                    bqc, bkc, bvc = cst[3][:], cst[4][:], cst[5][:]
                else:
                    bqc = bkc = bvc = None

                # per-seq-block LN1 rows
                rs_row = xp.tile([1, S], F32, tag="rs_row")
                nmu_row = xp.tile([1, S], BF, tag="nmu_row")
                std_row = xp.tile([1, S], BF, tag="std_row") if qkv_bias else None
                rsb_row = xp.tile([1, S], BF, tag="rsb_row")
                a_b = xp.tile([128, S], BF, tag="a_b")
                rst = xp.tile([128, S // 128], F32, tag="rst")

                # ---- LN1 statistics (two waves of 2 seq blocks) ------
                with (
                    tc.tile_pool(name="stps", bufs=4, space="PSUM") as stps,
                    tc.tile_pool(name="sq", bufs=3) as sqp,
                    tc.tile_pool(name="rows", bufs=2) as rowp,
                ):
                    st = [stps.tile([2, 512], F32, tag="stmu", name=f"stmu{_s}")
                          for _s in range(SB)]
                    stm2 = [stps.tile([2, 512], F32, tag="stm2", name=f"stm2{_s}")
                            for _s in range(SB)]
                    for wave in range(2):
                        for f in range(FT):
                            wsl = slice(1024 * wave, 1024 * wave + 1024)
                            x2 = sqp.tile([128, 1024], BF, tag="x2")
                            nc.vector.tensor_tensor(x2[:], x_sb[:, f, wsl],
                                                    x_sb[:, f, wsl], ALU.mult)
                            for si in range(2):
                                s = 2 * wave + si
                                sl = bass.ts(s, 512)
                                s2 = bass.ts(si, 512)
                                nc.tensor.matmul(st[s][0:1, :], recipd[:],
                                                 x_sb[:, f, sl],
                                                 start=(f == 0),
                                                 stop=(f == FT - 1))
                                nc.tensor.matmul(stm2[s][0:1, :], recipd[:],
                                                 x2[:, s2],
                                                 start=(f == 0),
                                                 stop=(f == FT - 1))
                        # postproc for this wave's two blocks
                        for si in range(2):
                            s = 2 * wave + si
                            sl = bass.ts(s, 512)
                            mu_r = rowp.tile([1, 512], F32, tag="mu_r")
                            m2_r = rowp.tile([1, 512], F32, tag="m2_r")
                            var_r = rowp.tile([1, 512], F32, tag="var_r")
                            lnv_r = rowp.tile([1, 512], F32, tag="lnv_r")
                            nc.scalar.copy(mu_r[:], st[s][0:1, :])
                            nc.scalar.copy(m2_r[:], stm2[s][0:1, :])
                            nc.vector.tensor_tensor(var_r[:], mu_r[:],
                                                    mu_r[:], ALU.mult)
                            nc.vector.scalar_tensor_tensor(
                                var_r[:], m2_r[:], EPS,
                                var_r[:], op0=ALU.add, op1=ALU.subtract)
                            nc.vector.tensor_scalar(
                                out=nmu_row[:, sl], in0=mu_r[:],
                                scalar1=-1.0, scalar2=None, op0=ALU.mult)
                            nc.scalar.activation(lnv_r[:], var_r[:], AF.Ln)
                            nc.scalar.activation(rs_row[:, sl], lnv_r[:],
                                                 AF.Exp, scale=-0.5)
                            nc.scalar.copy(rsb_row[:, sl], rs_row[:, sl])
                            if qkv_bias:
                                nc.scalar.activation(std_row[:, sl], lnv_r[:],
                                                     AF.Exp, scale=0.5)
                            nc.gpsimd.partition_broadcast(a_b[:, sl],
                                                          rsb_row[:, sl])
                # rs for v-scale: roundtrip to get [128, 16] per-block scalars
                drs = dramp.tile([1, S], F32, tag="drs")
                nc.sync.dma_start(out=drs[:], in_=rs_row[:])
                nc.sync.dma_start(
                    out=rst[:],
                    in_=drs[:].rearrange("o (t p) -> (o p) t", p=128))

                # ---- projections -----------------------------------
                with tc.tile_pool(name="prps", bufs=3, space="PSUM") as prps:
                    for (w_sb, wsum, bc, o_sb) in ((wq_sb, wqs, bqc, qt_sb),
                                                   (wk_sb, wks, bkc, kt_sb)):
                        for dd in range(2):
                            dsl = bass.ts(dd, 128)
                            for s in range(SB):
                                sl = bass.ts(s, 512)
                                ps = prps.tile([128, 512], F32, tag="pp")
                                for f in range(FT):
                                    nc.tensor.matmul(ps[:], w_sb[:, dd, f],
                                                     x_sb[:, f, sl],
                                                     start=(f == 0), stop=False)
                                nc.tensor.matmul(ps[:], wsum[0:1, dsl],
                                                 nmu_row[:, sl], start=False,
                                                 stop=not qkv_bias)
                                if qkv_bias:
                                    nc.tensor.matmul(ps[:], bc[0:1, dsl],
                                                     std_row[:, sl],
                                                     start=False, stop=True)
                                nc.vector.tensor_tensor(o_sb[:, dd, sl], ps[:],
                                                        a_b[:, sl], ALU.mult)
                    for i in range(16):
                        rl = bass.ts(i, 128)
                        ps = prps.tile([128, DC], F32, tag="pv")
                        for f in range(FT):
                            nc.tensor.matmul(ps[:], x_sb[:, f, rl], wv_sb[:, f],
                                             start=(f == 0), stop=False)
                        nc.tensor.matmul(ps[:], nmu_row[:, rl], wvs,
                                         start=False, stop=not qkv_bias)
                        if qkv_bias:
                            nc.tensor.matmul(ps[:], std_row[:, rl], bvc,
                                             start=False, stop=True)
                        nc.vector.tensor_scalar(
                            out=v_sb[:, i, :, 0:64],
                            in0=ps[:].rearrange("p (h e) -> p h e", h=HC),
                            scalar1=rst[:, i:i + 1], scalar2=None,
                            op0=ALU.mult)
            # x pool closed

            # ---- attention + chunked AllToAll ----------------------
            with (
                tc.tile_pool(name="attp", bufs=1) as atp_a,
                tc.tile_pool(name="scps", bufs=2, space="PSUM") as scps,
                tc.tile_pool(name="pvps", bufs=4, space="PSUM") as pvps,
                tc.tile_pool(name="exps", bufs=6) as expp,
                tc.tile_pool(name="rcps", bufs=3) as rcpp,
            ):
                attnt_sb = atp_a.tile([128, 2, S], FP8, tag="attnt")
                mask4 = atp_a.tile([128, 4, 512], BF, tag="mask4")
                nc.sync.dma_start(out=mask4[:],
                                  in_=d["mask4"].ap().rearrange("j p c -> p j c"))
                # prefetch stage-B weights while attention computes
                # (w1 allocated here so it reuses the freed x-pool space)
                w1_sb = wbig.tile([128, MT, FT, 128], BF, tag="w1")
                nc.sync.dma_start(out=wo_sb[:], in_=d["wo"].ap())
                for m in range(MT):
                    nc.sync.dma_start(out=w1_sb[:, m], in_=d["w1"].ap()[m])
                nc.sync.dma_start(out=bias_sb[:, 0:MT], in_=d["b1t"].ap())
                nc.sync.dma_start(out=bias_sb[:, MT:MT + FT], in_=d["b2t"].ap())
                nc.sync.dma_start(out=xres_sb[:], in_=d["xres"].ap())
                for hp in range(2):
                    heads = (2 * hp, 2 * hp + 1)
                    for qi in range(SB):
                        qsl = bass.ts(qi, 512)
                        nki = 4 * qi + 4
                        pv = {h: pvps.tile([65, 512], F32, tag="pv",
                                           name=f"pv{h}_{qi}") for h in heads}
                        for ki in range(nki):
                            # both heads' score matmuls run concurrently in
                            # distinct 64-row PE strips (row tiling)
                            sc = scps.tile([128, 2, 512], F32, tag="sc")
                            for u in range(2):
                                h = heads[u]
                                hs = slice(64 * u, 64 * u + 64)
                                nc.tensor.matmul(
                                    sc[:, u],
                                    kt_sb[hs, h // 2, bass.ts(ki, 128)],
                                    qt_sb[hs, h // 2, qsl],
                                    start=True, stop=True)
                            ex = expp.tile([128, 2, 512], BF, tag="ex")
                            nc.scalar.activation(
                                ex[:].rearrange("p u s -> p (u s)"),
                                sc[:].rearrange("p u s -> p (u s)"),
                                AF.Exp)
                            rel = 128 * ki - 512 * qi
                            if rel >= 0:
                                mw = rel + 128
                                for u in range(2):
                                    nc.vector.tensor_tensor(
                                        ex[:, u, 0:mw], ex[:, u, 0:mw],
                                        mask4[:, rel // 128, 0:mw],
                                        ALU.mult)
                            for u in range(2):
                                h = heads[u]
                                nc.tensor.matmul(pv[h][:],
                                                 v_sb[:, ki, h, :],
                                                 ex[:, u],
                                                 start=(ki == 0),
                                                 stop=(ki == nki - 1))
                        for h in heads:
                            hb = 64 * (h % 2)
                            rcp = rcpp.tile([1, 512], F32, tag="rcp")
                            rcpb = rcpp.tile([64, 512], F32, tag="rcpb")
                            nc.vector.reciprocal(rcp[:], pv[h][64:65, :])
                            nc.gpsimd.partition_broadcast(rcpb[:], rcp[:])
                            nc.vector.tensor_tensor(
                                attnt_sb[hb:hb + 64, hp, qsl],
                                pv[h][0:64, :], rcpb[:], ALU.mult)
                    # this head-pair is done for all S: ship its AllGather
                    nc.sync.dma_start(out=ag_in[hp][:],
                                      in_=attnt_sb[:, hp, :])
                    nc.gpsimd.collective_compute(
                        "AllGather", ALU.bypass, replica_groups=REPLICA_GROUPS,
                        ins=[ag_in[hp].opt()], outs=[ag_out[hp].opt()])
        # stage-A pools closed

        # ============================ stage B ============================
        creg = nc.alloc_registers(f"colsel_r_{nc.next_id()}")
        nc.regs_load(creg, d["colsel"].ap()[0:1, 0:1])
        colsv = nc.snap(creg, donate=True, min_val=0, max_val=S - RC)

        with tc.tile_pool(name="persb", bufs=1) as pb:
            h_sb = pb.tile([128, FT, RC], BF, tag="h")
            st2 = pb.tile([1, 4 * RC], F32, tag="st2")
            MU2, VAR2, LNV2 = 0, 2 * RC, 3 * RC
            rs2_row = pb.tile([1, RC], F32, tag="rs2_row")
            l2a = pb.tile([1, RC], BF, tag="l2a")
            l2b = pb.tile([1, RC], BF, tag="l2b")
            l2a_b = pb.tile([128, RC], BF, tag="l2a_b")
            l2b_b = pb.tile([128, RC], BF, tag="l2b_b")

            # ---- Wo (two passes over AllGather chunks) + residual ----
            with (
                tc.tile_pool(name="atin", bufs=1) as atp,
                tc.tile_pool(name="h0p", bufs=1) as h0p,
                tc.tile_pool(name="wops", bufs=3, space="PSUM") as wops,
                tc.tile_pool(name="st2ps", bufs=1, space="PSUM") as st2ps,
                tc.tile_pool(name="sq2", bufs=2) as sq2p,
            ):
                stp = st2ps.tile([2, RC], F32, tag="st2p")
                stp2 = st2ps.tile([2, RC], F32, tag="st2p2")
                at_f8 = atp.tile([128, 2, TP, RC], FP8, tag="at_f8")
                at_in = atp.tile([128, 2, TP, RC], BF, tag="at_in")
                h0_sb = h0p.tile([128, FT, RC], F32, tag="h0")
                warm2 = wops.tile([2, RC], F32, tag="warm2")
                wov = wo_sb[:, 0, 0].rearrange("p f c -> p (f c)")
                for j in range(2):
                    bo_view = ag_out[j][:].rearrange("(g p) s -> p g s", p=128)
                    for g in range(TP):
                        nc.sync.dma_start(
                            out=at_f8[:, j, g, :],
                            in_=bo_view[:, g, bass.ds(colsv, RC)])
                        nc.vector.tensor_scalar(
                            out=at_in[:, j, g, :], in0=at_f8[:, j, g, :],
                            scalar1=1.0, scalar2=None, op0=ALU.mult)
                    if j == 0:
                        # keep PE warm while the second AllGather lands
                        for i in range(16):
                            nc.tensor.matmul(warm2[0:1, :], recipd[:],
                                             wov[:, 0:RC],
                                             start=True, stop=True)
                    for dd in range(FT):
                        ps = wops.tile([128, RC], F32, tag="wops")
                        for g in range(TP):
                            nc.tensor.matmul(ps[:], wo_sb[:, j, g, dd],
                                             at_in[:, j, g, :],
                                             start=(g == 0), stop=(g == TP - 1))
                        if j == 0:
                            nc.vector.tensor_tensor(h0_sb[:, dd, :], ps[:],
                                                    xres_sb[:, dd, :], ALU.add)
                        else:
                            nc.vector.tensor_tensor(h_sb[:, dd, :], ps[:],
                                                    h0_sb[:, dd, :], ALU.add)
                            h2 = sq2p.tile([128, RC], BF, tag="h2")
                            nc.vector.tensor_tensor(h2[:], h_sb[:, dd],
                                                    h_sb[:, dd], ALU.mult)
                            nc.tensor.matmul(stp[0:1, :], recipd[:],
                                             h_sb[:, dd, :],
                                             start=(dd == 0),
                                             stop=(dd == FT - 1))
                            nc.tensor.matmul(stp2[0:1, :], recipd[:], h2[:],
                                             start=(dd == 0),
                                             stop=(dd == FT - 1))
                nc.scalar.copy(st2[:, MU2:MU2 + RC], stp[0:1, :])
                nc.scalar.copy(st2[:, MU2 + RC:MU2 + 2 * RC], stp2[0:1, :])

            # ---- LN2 + FFN -----------------------------------------
            with (
                tc.tile_pool(name="hnp", bufs=1) as hnp,
                tc.tile_pool(name="ap_", bufs=1) as ap_,
                tc.tile_pool(name="w2s", bufs=3) as w2p,
                tc.tile_pool(name="outs", bufs=2) as outsp,
                tc.tile_pool(name="f1ps", bufs=3, space="PSUM") as f1ps,
                tc.tile_pool(name="f2ps", bufs=2, space="PSUM") as f2ps,
            ):
                nc.vector.tensor_tensor(st2[:, VAR2:VAR2 + RC],
                                        st2[:, MU2:MU2 + RC],
                                        st2[:, MU2:MU2 + RC], ALU.mult)
                nc.vector.scalar_tensor_tensor(st2[:, VAR2:VAR2 + RC],
                                               st2[:, MU2 + RC:MU2 + 2 * RC],
                                               EPS,
                                               st2[:, VAR2:VAR2 + RC],
                                               op0=ALU.add, op1=ALU.subtract)
                nc.scalar.activation(st2[:, LNV2:LNV2 + RC],
                                     st2[:, VAR2:VAR2 + RC], AF.Ln)
                nc.scalar.activation(rs2_row[:], st2[:, LNV2:LNV2 + RC],
                                     AF.Exp, scale=-0.5)
                nc.scalar.copy(l2a[:], rs2_row[:])
                nc.vector.scalar_tensor_tensor(l2b[:], st2[:, MU2:MU2 + RC],
                                               -1.0, rs2_row[:],
                                               op0=ALU.mult, op1=ALU.mult)
                nc.gpsimd.partition_broadcast(l2a_b[:], l2a[:])
                nc.gpsimd.partition_broadcast(l2b_b[:], l2b[:])

                hn_sb = hnp.tile([128, FT, RC], BF, tag="hn")
                for f in range(FT):
                    nc.vector.tensor_tensor(hn_sb[:, f, :], h_sb[:, f, :],
                                            l2a_b[:], ALU.mult)
                    nc.vector.tensor_add(hn_sb[:, f, :], hn_sb[:, f, :],
                                         l2b_b[:])

                a_sb = ap_.tile([128, MT, RC], BF, tag="a")
                for m in range(MT):
                    ps = f1ps.tile([128, RC], F32, tag="f1")
                    for f in range(FT):
                        nc.tensor.matmul(ps[:], w1_sb[:, m, f, :],
                                         hn_sb[:, f, :],
                                         start=(f == 0), stop=(f == FT - 1))
                    nc.scalar.activation(a_sb[:, m, :], ps[:], AF.Relu,
                                         bias=bias_sb[:, m:m + 1])

                for dd in range(FT):
                    w2d = w2p.tile([128, MT, 128], BF, tag="w2d")
                    nc.sync.dma_start(out=w2d[:], in_=d["w2"].ap()[dd])
                    ps = f2ps.tile([128, RC], F32, tag="f2")
                    for t in range(MT):
                        nc.tensor.matmul(ps[:], w2d[:, t, :], a_sb[:, t, :],
                                         start=(t == 0), stop=(t == MT - 1))
                    o_t = outsp.tile([128, RC], F32, tag="ot")
                    nc.vector.scalar_tensor_tensor(
                        o_t[:], ps[:],
                        bias_sb[:, MT + dd:MT + dd + 1],
                        h_sb[:, dd, :], op0=ALU.add, op1=ALU.add)
                    nc.sync.dma_start(out=d["out"].ap()[:, dd], in_=o_t[:])


# ----------------------------------------------------------------------
# host side
# ----------------------------------------------------------------------

def make_in_maps(x, mask, Wq, Wk, Wv, Wo, w1, b1, w2, b2, g1, be1, g2, be2):
    """Build the 8 per-core input maps from the full inputs."""
    f32 = np.float32
    x = np.asarray(x, f32)
    mask = np.asarray(mask)
    Wq, Wk, Wv, Wo = (np.asarray(t, f32) for t in (Wq, Wk, Wv, Wo))
    w1, b1, w2, b2 = (np.asarray(t, f32) for t in (w1, b1, w2, b2))
    g1, be1, g2, be2 = (np.asarray(t, f32) for t in (g1, be1, g2, be2))

    Wq_s = g1[:, None] * Wq / np.sqrt(np.float32(DH))
    Wk_s = g1[:, None] * Wk
    Wv_s = g1[:, None] * Wv
    bq_full = (be1 @ Wq) / np.sqrt(np.float32(DH))
    bk_full = be1 @ Wk
    bv_full = be1 @ Wv
    w1_s = g2[:, None] * w1
    b1_s = b1 + be2 @ w1
    m2d = np.asarray(mask[0, 0], bool)
    mask4 = np.stack([m2d[0:512, 128 * j:128 * j + 128].T.astype(f32)
                      for j in range(4)]).astype(NPBF)
    recipd = np.full((128, 1), 1.0 / D, NPBF)
    ones64 = np.ones((128, 16, HC, 1), NPBF)
    b1t = np.ascontiguousarray(b1_s.reshape(MT, 128).T)
    b2t = np.ascontiguousarray(b2.reshape(FT, 128).T)
    # wo[p, j, g, dd, c] = Wo[(2g+j)*128+p, dd*128+c]
    wo_p = np.ascontiguousarray(
        Wo.reshape(TP, 2, 128, FT, 128).transpose(2, 1, 0, 3, 4)).astype(NPBF)
    w1_p = np.ascontiguousarray(
        w1_s.reshape(FT, 128, MT, 128).transpose(2, 1, 0, 3)).astype(NPBF)
    w2_p = np.ascontiguousarray(
        w2.reshape(MT, 128, FT, 128).transpose(2, 1, 0, 3)).astype(NPBF)

    in_maps = []
    for c in range(N_CORES):
        g, r = divmod(c, TP)
        xT = np.ascontiguousarray(x[g].T)                       # [D, S]
        xt = np.ascontiguousarray(
            xT.reshape(FT, 128, S).transpose(1, 0, 2)).astype(NPBF)
        xres = np.ascontiguousarray(
            xT[:, RC * r:RC * (r + 1)].reshape(FT, 128, RC)
            .transpose(1, 0, 2)).astype(NPBF)
        sh = slice(DC * r, DC * (r + 1))
        wq_c = np.ascontiguousarray(
            Wq_s[:, sh].reshape(FT, 128, 2, 128).transpose(2, 1, 0, 3)
        ).astype(NPBF)
        wk_c = np.ascontiguousarray(
            Wk_s[:, sh].reshape(FT, 128, 2, 128).transpose(2, 1, 0, 3)
        ).astype(NPBF)
        wv_c = np.ascontiguousarray(
            Wv_s[:, sh].reshape(FT, 128, DC).transpose(1, 0, 2)).astype(NPBF)
        qkvc = np.stack([Wq_s[:, sh].sum(0), Wk_s[:, sh].sum(0),
                         Wv_s[:, sh].sum(0), bq_full[sh], bk_full[sh],
                         bv_full[sh]]).astype(NPBF)
        in_maps.append({
            "xt": xt, "xres": xres, "wq": wq_c, "wk": wk_c, "wv": wv_c,
            "qkvc": qkvc, "wo": wo_p, "w1": w1_p, "b1t": b1t, "w2": w2_p,
            "b2t": b2t, "mask4": mask4, "recipd": recipd,
            "ones64": ones64,
            "colsel": np.array([[RC * r]], np.uint32),
        })
    return in_maps


def assemble_output(results):
    """[8 x {out: [128, FT, RC]}] -> [B, S, D] float32."""
    out = np.empty((B, S, D), np.float32)
    for c in range(N_CORES):
        g, r = divmod(c, TP)
        ot = results[c]["out"].transpose(1, 0, 2).reshape(D, RC)  # [D, RC]
        out[g, RC * r:RC * (r + 1), :] = ot.T
    return out


_nc_cache = {}


def get_nc(repeat=1, qkv_bias=False, **_ignored):
    key = (repeat, qkv_bias)
    if key not in _nc_cache:
        _nc_cache[key] = build(repeat=repeat, qkv_bias=qkv_bias)
    return _nc_cache[key]


def kernel(**inputs):
    qkv_bias = bool(np.any(np.asarray(inputs["be1"], np.float32)))
    nc = get_nc(qkv_bias=qkv_bias)
    in_maps = make_in_maps(**inputs)
    res = run_bass_kernel_spmd(nc, in_maps, core_ids=list(range(N_CORES)))
    return assemble_output(res.results)


# revision 21
# speedup vs baseline: 1.1012x; 1.0737x over previous
"""Trainium2 Bass kernel: pre-norm decoder block (B=2, S=2048, D=1024, H=16, DFF=4096).

Sharding: 8 cores = 2 data-parallel groups (one per batch) x 4 tensor-parallel
ranks. Within a group, attention is head-sharded (4 heads/core, Megatron-style
column-parallel QKV). The head-sharded attention output is redistributed with
two chunked AllToAlls (one per head-pair) so each rank ends up with the full
1024 attention features for its own 512 sequence positions; the rest of the
block (Wo + residual + LN2 + FFN + residual) runs sequence-sharded with the
full Wo/w1/w2, so no further collectives are needed. The AllToAll moves 4x
fewer bytes than the equivalent AllGather and each chunk overlaps with either
the second half of attention or the first Wo pass.

All matmul operands are bf16 (fp32 PSUM accumulation): halves HBM traffic,
SBUF footprint and collective payload, enables the PE fast-weight-load path
and DVE 2x/4x modes. On-chip activations live in transposed layout
([feature, seq]) so no on-chip transposes are needed. LayerNorm statistics
are ones-vector matmuls on the tensor engine (partition-axis reduction in
this layout), pipelined in two seq-halves; the per-position LN affine is
folded into the projections via extra contraction rows.
"""

import numpy as np
import ml_dtypes

import concourse.bass as bass
import concourse.mybir as mybir
import concourse.tile as tile
from concourse import bacc
from concourse.bass_utils import run_bass_kernel_spmd

BF = mybir.dt.bfloat16
FP8 = mybir.dt.float8e4
F32 = mybir.dt.float32
AF = mybir.ActivationFunctionType
ALU = mybir.AluOpType
NPBF = ml_dtypes.bfloat16

B, S, D, H, DFF = 2, 2048, 1024, 16, 4096
DH = D // H
EPS = 1e-5

N_CORES = 8
TP = 4                    # tensor-parallel ranks per group
HC = H // TP              # heads per core
DC = HC * DH              # head features per core
RC = S // TP              # seq rows per core in stage B
FT = D // 128             # feature tiles
SB = S // 512             # 512-wide seq blocks
MT = DFF // 128           # dff tiles
REPLICA_GROUPS = [[0, 1, 2, 3], [4, 5, 6, 7]]


def build(repeat=1, qkv_bias=False):
    nc = bacc.Bacc("TRN2", target_bir_lowering=False, debug=False,
                   num_devices=N_CORES)

    d = {"qkv_bias": qkv_bias}
    d["xt"] = nc.dram_tensor("xt", [128, FT, S], BF, kind="ExternalInput")
    d["xres"] = nc.dram_tensor("xres", [128, FT, RC], BF, kind="ExternalInput")
    d["wq"] = nc.dram_tensor("wq", [2, 128, FT, 128], BF, kind="ExternalInput")
    d["wk"] = nc.dram_tensor("wk", [2, 128, FT, 128], BF, kind="ExternalInput")
    d["wv"] = nc.dram_tensor("wv", [128, FT, DC], BF, kind="ExternalInput")
    d["qkvc"] = nc.dram_tensor("qkvc", [6, DC], BF, kind="ExternalInput")
    # wo indexed [p, j, g, dd, c]: contraction tile f = 2g + j
    d["wo"] = nc.dram_tensor("wo", [128, 2, TP, FT, 128], BF, kind="ExternalInput")
    d["w1"] = nc.dram_tensor("w1", [MT, 128, FT, 128], BF, kind="ExternalInput")
    d["b1t"] = nc.dram_tensor("b1t", [128, MT], F32, kind="ExternalInput")
    d["w2"] = nc.dram_tensor("w2", [FT, 128, MT, 128], BF, kind="ExternalInput")
    d["b2t"] = nc.dram_tensor("b2t", [128, FT], F32, kind="ExternalInput")
    d["mask4"] = nc.dram_tensor("mask4", [4, 128, 512], BF, kind="ExternalInput")
    d["recipd"] = nc.dram_tensor("recipd", [128, 1], BF, kind="ExternalInput")
    d["ones64"] = nc.dram_tensor("ones64", [128, 16, HC, 1], BF, kind="ExternalInput")
    d["colsel"] = nc.dram_tensor("colsel", [1, 1], mybir.dt.uint32,
                                 kind="ExternalInput")
    d["out"] = nc.dram_tensor("out", [128, FT, RC], F32, kind="ExternalOutput")

    with tile.TileContext(nc) as tc:
        for _ in range(repeat):
            _emit(nc, tc, d)

    nc.compile()
    return nc


def _emit(nc, tc, d):
    qkv_bias = d["qkv_bias"]
    with (
        tc.tile_pool(name="dram", bufs=1, space="DRAM") as dramp,
        tc.tile_pool(name="wbig", bufs=1) as wbig,
        tc.tile_pool(name="outer", bufs=1) as outp,
    ):
        # DRAM bounce buffers for the two AllGather chunks (one per head pair)
        ag_in = [dramp.tile([128, S], FP8, tag=f"ag_in{j}",
                            name=f"ag_in{j}") for j in range(2)]
        ag_out = [dramp.tile([TP * 128, S], FP8, tag=f"ag_out{j}",
                             name=f"ag_out{j}") for j in range(2)]
        recipd = outp.tile([128, 1], BF, tag="recipd")
        nc.sync.dma_start(out=recipd[:], in_=d["recipd"].ap())
        # FFN / Wo weights prefetched during attention; consumed in stage B.
        wo_sb = wbig.tile([128, 2, TP, FT, 128], BF, tag="wo")
        bias_sb = outp.tile([128, MT + FT], F32, tag="bias")
        xres_sb = outp.tile([128, FT, RC], BF, tag="xres")

        # ============================ stage A ============================
        with tc.tile_pool(name="persa", bufs=1) as pa:
            qt_sb = pa.tile([128, 2, S], BF, tag="qt")
            kt_sb = pa.tile([128, 2, S], BF, tag="kt")
            v_sb = pa.tile([128, 16, HC, 65], BF, tag="v")
            nc.sync.dma_start(out=v_sb[:, :, :, 64:65], in_=d["ones64"].ap())

            with tc.tile_pool(name="xpool", bufs=1) as xp:
                x_sb = xp.tile([128, FT, S], BF, tag="x")
                wq_sb = xp.tile([128, 2, FT, 128], BF, tag="wq")
                wk_sb = xp.tile([128, 2, FT, 128], BF, tag="wk")
                wv_sb = xp.tile([128, FT, DC], BF, tag="wv")
                nc.sync.dma_start(out=wq_sb[:, 0], in_=d["wq"].ap()[0])
                for quarter in range(4):
                    hl = slice(512 * quarter, 512 * quarter + 512)
                    for f in range(FT):
                        nc.sync.dma_start(out=x_sb[:, f, hl],
                                          in_=d["xt"].ap()[:, f, hl])
                nc.sync.dma_start(out=wq_sb[:, 1], in_=d["wq"].ap()[1])
                for dd in range(2):
                    nc.sync.dma_start(out=wk_sb[:, dd], in_=d["wk"].ap()[dd])
                nc.sync.dma_start(out=wv_sb[:], in_=d["wv"].ap())
                # HAM warmup: dummy matmuls over the resident wq tile keep
                # the PE busy (and unthrottled) while x streams from HBM
                with tc.tile_pool(name="warm", bufs=1, space="PSUM") as wmp:
                    warm = wmp.tile([2, 512], F32, tag="warm")
                    wqv = wq_sb[:, 0].rearrange("p f c -> p (f c)")
                    for i in range(20):
                        nc.tensor.matmul(warm[0:1, :],
                                         recipd[:], wqv[:, 0:512],
                                         start=True, stop=True)
                ncst = 6 if qkv_bias else 3
                cst = [xp.tile([1, DC], BF, tag=f"qkvc{i}", name=f"qkvc{i}")
                       for i in range(ncst)]
                for i in range(ncst):
                    nc.sync.dma_start(out=cst[i][:], in_=d["qkvc"].ap()[i:i + 1, :])
                wqs, wks, wvs = cst[0][:], cst[1][:], cst[2][:]
                if qkv_bias:
                    bqc, bkc, bvc = cst[3][:], cst[4][:], cst[5][:]
                else:
                    bqc = bkc = bvc = None

                # per-seq-block LN1 rows
                rs_row = xp.tile([1, S], F32, tag="rs_row")
                nmu_row = xp.tile([1, S], BF, tag="nmu_row")
                std_row = xp.tile([1, S], BF, tag="std_row") if qkv_bias else None
                rsb_row = xp.tile([1, S], BF, tag="rsb_row")
                a_b = xp.tile([128, S], BF, tag="a_b")
                rst = xp.tile([128, S // 128], F32, tag="rst")

                # ---- LN1 statistics (two waves of 2 seq blocks) ------
                with (
                    tc.tile_pool(name="stps", bufs=4, space="PSUM") as stps,
                    tc.tile_pool(name="sq", bufs=3) as sqp,
                    tc.tile_pool(name="rows", bufs=2) as rowp,
                ):
                    st = [stps.tile([2, 512], F32, tag="stmu", name=f"stmu{_s}")
                          for _s in range(SB)]
                    stm2 = [stps.tile([2, 512], F32, tag="stm2", name=f"stm2{_s}")
                            for _s in range(SB)]
                    for wave in range(2):
                        for f in range(FT):
                            wsl = slice(1024 * wave, 1024 * wave + 1024)
                            x2 = sqp.tile([128, 1024], BF, tag="x2")
                            nc.vector.tensor_tensor(x2[:], x_sb[:, f, wsl],
                                                    x_sb[:, f, wsl], ALU.mult)
                            for si in range(2):
                                s = 2 * wave + si
                                sl = bass.ts(s, 512)
                                s2 = bass.ts(si, 512)
                                nc.tensor.matmul(st[s][0:1, :], recipd[:],
                                                 x_sb[:, f, sl],
                                                 start=(f == 0),
                                                 stop=(f == FT - 1))
                                nc.tensor.matmul(stm2[s][0:1, :], recipd[:],
                                                 x2[:, s2],
                                                 start=(f == 0),
                                                 stop=(f == FT - 1))
                        # postproc for this wave's two blocks
                        for si in range(2):
                            s = 2 * wave + si
                            sl = bass.ts(s, 512)
                            mu_r = rowp.tile([1, 512], F32, tag="mu_r")
                            m2_r = rowp.tile([1, 512], F32, tag="m2_r")
                            var_r = rowp.tile([1, 512], F32, tag="var_r")
                            lnv_r = rowp.tile([1, 512], F32, tag="lnv_r")
                            nc.scalar.copy(mu_r[:], st[s][0:1, :])
                            nc.scalar.copy(m2_r[:], stm2[s][0:1, :])
                            nc.vector.tensor_tensor(var_r[:], mu_r[:],
                                                    mu_r[:], ALU.mult)
                            nc.vector.scalar_tensor_tensor(
                                var_r[:], m2_r[:], EPS,
                                var_r[:], op0=ALU.add, op1=ALU.subtract)
                            nc.vector.tensor_scalar(
                                out=nmu_row[:, sl], in0=mu_r[:],
                                scalar1=-1.0, scalar2=None, op0=ALU.mult)
                            nc.scalar.activation(lnv_r[:], var_r[:], AF.Ln)
                            nc.scalar.activation(rs_row[:, sl], lnv_r[:],
                                                 AF.Exp, scale=-0.5)
                            nc.scalar.copy(rsb_row[:, sl], rs_row[:, sl])
                            if qkv_bias:
                                nc.scalar.activation(std_row[:, sl], lnv_r[:],
                                                     AF.Exp, scale=0.5)
                            nc.gpsimd.partition_broadcast(a_b[:, sl],
                                                          rsb_row[:, sl])
                # rs for v-scale: roundtrip to get [128, 16] per-block scalars
                drs = dramp.tile([1, S], F32, tag="drs")
                nc.sync.dma_start(out=drs[:], in_=rs_row[:])
                nc.sync.dma_start(
                    out=rst[:],
                    in_=drs[:].rearrange("o (t p) -> (o p) t", p=128))

                # ---- projections -----------------------------------
                with tc.tile_pool(name="prps", bufs=3, space="PSUM") as prps:
                    for (w_sb, wsum, bc, o_sb) in ((wq_sb, wqs, bqc, qt_sb),
                                                   (wk_sb, wks, bkc, kt_sb)):
                        for dd in range(2):
                            dsl = bass.ts(dd, 128)
                            for s in range(SB):
                                sl = bass.ts(s, 512)
                                ps = prps.tile([128, 512], F32, tag="pp")
                                for f in range(FT):
                                    nc.tensor.matmul(ps[:], w_sb[:, dd, f],
                                                     x_sb[:, f, sl],
                                                     start=(f == 0), stop=False)
                                nc.tensor.matmul(ps[:], wsum[0:1, dsl],
                                                 nmu_row[:, sl], start=False,
                                                 stop=not qkv_bias)
                                if qkv_bias:
                                    nc.tensor.matmul(ps[:], bc[0:1, dsl],
                                                     std_row[:, sl],
                                                     start=False, stop=True)
                                nc.vector.tensor_tensor(o_sb[:, dd, sl], ps[:],
                                                        a_b[:, sl], ALU.mult)
                    for i in range(16):
                        rl = bass.ts(i, 128)
                        ps = prps.tile([128, DC], F32, tag="pv")
                        for f in range(FT):
                            nc.tensor.matmul(ps[:], x_sb[:, f, rl], wv_sb[:, f],
                                             start=(f == 0), stop=False)
                        nc.tensor.matmul(ps[:], nmu_row[:, rl], wvs,
                                         start=False, stop=not qkv_bias)
                        if qkv_bias:
                            nc.tensor.matmul(ps[:], std_row[:, rl], bvc,
                                             start=False, stop=True)
                        nc.vector.tensor_scalar(
                            out=v_sb[:, i, :, 0:64],
                            in0=ps[:].rearrange("p (h e) -> p h e", h=HC),
                            scalar1=rst[:, i:i + 1], scalar2=None,
                            op0=ALU.mult)
            # x pool closed

            # ---- attention + chunked AllToAll ----------------------
            with (
                tc.tile_pool(name="attp", bufs=1) as atp_a,
                tc.tile_pool(name="scps", bufs=2, space="PSUM") as scps,
                tc.tile_pool(name="pvps", bufs=4, space="PSUM") as pvps,
                tc.tile_pool(name="exps", bufs=6) as expp,
                tc.tile_pool(name="rcps", bufs=3) as rcpp,
            ):
                attnt_sb = atp_a.tile([128, 2, S], FP8, tag="attnt")
                mask4 = atp_a.tile([128, 4, 512], BF, tag="mask4")
                nc.sync.dma_start(out=mask4[:],
                                  in_=d["mask4"].ap().rearrange("j p c -> p j c"))
                # prefetch stage-B weights while attention computes
                # (w1 allocated here so it reuses the freed x-pool space)
                w1_sb = wbig.tile([128, MT, FT, 128], BF, tag="w1")
                nc.sync.dma_start(out=wo_sb[:], in_=d["wo"].ap())
                for m in range(MT):
                    nc.sync.dma_start(out=w1_sb[:, m], in_=d["w1"].ap()[m])
                nc.sync.dma_start(out=bias_sb[:, 0:MT], in_=d["b1t"].ap())
                nc.sync.dma_start(out=bias_sb[:, MT:MT + FT], in_=d["b2t"].ap())
                nc.sync.dma_start(out=xres_sb[:], in_=d["xres"].ap())
                for hp in range(2):
                    heads = (2 * hp, 2 * hp + 1)
                    for qi in range(SB):
                        qsl = bass.ts(qi, 512)
                        nki = 4 * qi + 4
                        pv = {h: pvps.tile([65, 512], F32, tag="pv",
                                           name=f"pv{h}_{qi}") for h in heads}
                        for ki in range(nki):
                            # both heads' score matmuls run concurrently in
                            # distinct 64-row PE strips (row tiling)
                            sc = scps.tile([128, 2, 512], F32, tag="sc")
                            for u in range(2):
                                h = heads[u]
                                hs = slice(64 * u, 64 * u + 64)
                                nc.tensor.matmul(
                                    sc[:, u],
                                    kt_sb[hs, h // 2, bass.ts(ki, 128)],
                                    qt_sb[hs, h // 2, qsl],
                                    start=True, stop=True)
                            ex = expp.tile([128, 2, 512], BF, tag="ex")
                            nc.scalar.activation(
                                ex[:].rearrange("p u s -> p (u s)"),
                                sc[:].rearrange("p u s -> p (u s)"),
                                AF.Exp)
                            rel = 128 * ki - 512 * qi
                            if rel >= 0:
                                mw = rel + 128
                                for u in range(2):
                                    nc.vector.tensor_tensor(
                                        ex[:, u, 0:mw], ex[:, u, 0:mw],
                                        mask4[:, rel // 128, 0:mw],
                                        ALU.mult)
                            for u in range(2):
                                h = heads[u]
                                nc.tensor.matmul(pv[h][:],
                                                 v_sb[:, ki, h, :],
                                                 ex[:, u],
                                                 start=(ki == 0),
                                                 stop=(ki == nki - 1))
                        for h in heads:
                            hb = 64 * (h % 2)
                            rcp = rcpp.tile([1, 512], F32, tag="rcp")
                            rcpb = rcpp.tile([64, 512], F32, tag="rcpb")
                            nc.vector.reciprocal(rcp[:], pv[h][64:65, :])
                            nc.gpsimd.partition_broadcast(rcpb[:], rcp[:])
                            nc.vector.tensor_tensor(
                                attnt_sb[hb:hb + 64, hp, qsl],
                                pv[h][0:64, :], rcpb[:], ALU.mult)
                    # this head-pair is done for all S: ship its AllGather
                    nc.sync.dma_start(out=ag_in[hp][:],
                                      in_=attnt_sb[:, hp, :])
                    nc.gpsimd.collective_compute(
                        "AllGather", ALU.bypass, replica_groups=REPLICA_GROUPS,
                        ins=[ag_in[hp].opt()], outs=[ag_out[hp].opt()])
        # stage-A pools closed

        # ============================ stage B ============================
        creg = nc.alloc_registers(f"colsel_r_{nc.next_id()}")
        nc.regs_load(creg, d["colsel"].ap()[0:1, 0:1])
        colsv = nc.snap(creg, donate=True, min_val=0, max_val=S - RC)

        with tc.tile_pool(name="persb", bufs=1) as pb:
            h_sb = pb.tile([128, FT, RC], BF, tag="h")
            st2 = pb.tile([1, 4 * RC], F32, tag="st2")
            MU2, VAR2, LNV2 = 0, 2 * RC, 3 * RC
            rs2_row = pb.tile([1, RC], F32, tag="rs2_row")
            l2a = pb.tile([1, RC], BF, tag="l2a")
            l2b = pb.tile([1, RC], BF, tag="l2b")
            l2a_b = pb.tile([128, RC], BF, tag="l2a_b")
            l2b_b = pb.tile([128, RC], BF, tag="l2b_b")

            # ---- Wo (two passes over AllGather chunks) + residual ----
            with (
                tc.tile_pool(name="atin", bufs=1) as atp,
                tc.tile_pool(name="h0p", bufs=1) as h0p,
                tc.tile_pool(name="wops", bufs=3, space="PSUM") as wops,
                tc.tile_pool(name="st2ps", bufs=1, space="PSUM") as st2ps,
                tc.tile_pool(name="sq2", bufs=2) as sq2p,
            ):
                stp = st2ps.tile([2, RC], F32, tag="st2p")
                stp2 = st2ps.tile([2, RC], F32, tag="st2p2")
                at_f8 = atp.tile([128, 2, TP, RC], FP8, tag="at_f8")
                at_in = atp.tile([128, 2, TP, RC], BF, tag="at_in")
                h0_sb = h0p.tile([128, FT, RC], F32, tag="h0")
                for j in range(2):
                    bo_view = ag_out[j][:].rearrange("(g p) s -> p g s", p=128)
                    for g in range(TP):
                        nc.sync.dma_start(
                            out=at_f8[:, j, g, :],
                            in_=bo_view[:, g, bass.ds(colsv, RC)])
                        nc.vector.tensor_scalar(
                            out=at_in[:, j, g, :], in0=at_f8[:, j, g, :],
                            scalar1=1.0, scalar2=None, op0=ALU.mult)

                    for dd in range(FT):
                        ps = wops.tile([128, RC], F32, tag="wops")
                        for g in range(TP):
                            nc.tensor.matmul(ps[:], wo_sb[:, j, g, dd],
                                             at_in[:, j, g, :],
                                             start=(g == 0), stop=(g == TP - 1))
                        if j == 0:
                            nc.vector.tensor_tensor(h0_sb[:, dd, :], ps[:],
                                                    xres_sb[:, dd, :], ALU.add)
                        else:
                            nc.vector.tensor_tensor(h_sb[:, dd, :], ps[:],
                                                    h0_sb[:, dd, :], ALU.add)
                            h2 = sq2p.tile([128, RC], BF, tag="h2")
                            nc.vector.tensor_tensor(h2[:], h_sb[:, dd],
                                                    h_sb[:, dd], ALU.mult)
                            nc.tensor.matmul(stp[0:1, :], recipd[:],
                                             h_sb[:, dd, :],
                                             start=(dd == 0),
                                             stop=(dd == FT - 1))
                            nc.tensor.matmul(stp2[0:1, :], recipd[:], h2[:],
                                             start=(dd == 0),
                                             stop=(dd == FT - 1))
                nc.scalar.copy(st2[:, MU2:MU2 + RC], stp[0:1, :])
                nc.scalar.copy(st2[:, MU2 + RC:MU2 + 2 * RC], stp2[0:1, :])

            # ---- LN2 + FFN -----------------------------------------
            with (
                tc.tile_pool(name="hnp", bufs=1) as hnp,
                tc.tile_pool(name="ap_", bufs=1) as ap_,
                tc.tile_pool(name="w2s", bufs=3) as w2p,
                tc.tile_pool(name="outs", bufs=2) as outsp,
                tc.tile_pool(name="f1ps", bufs=3, space="PSUM") as f1ps,
                tc.tile_pool(name="f2ps", bufs=2, space="PSUM") as f2ps,
            ):
                nc.vector.tensor_tensor(st2[:, VAR2:VAR2 + RC],
                                        st2[:, MU2:MU2 + RC],
                                        st2[:, MU2:MU2 + RC], ALU.mult)
                nc.vector.scalar_tensor_tensor(st2[:, VAR2:VAR2 + RC],
                                               st2[:, MU2 + RC:MU2 + 2 * RC],
                                               EPS,
                                               st2[:, VAR2:VAR2 + RC],
                                               op0=ALU.add, op1=ALU.subtract)
                nc.scalar.activation(st2[:, LNV2:LNV2 + RC],
                                     st2[:, VAR2:VAR2 + RC], AF.Ln)
                nc.scalar.activation(rs2_row[:], st2[:, LNV2:LNV2 + RC],
                                     AF.Exp, scale=-0.5)
                nc.scalar.copy(l2a[:], rs2_row[:])
                nc.vector.scalar_tensor_tensor(l2b[:], st2[:, MU2:MU2 + RC],
                                               -1.0, rs2_row[:],
                                               op0=ALU.mult, op1=ALU.mult)
                nc.gpsimd.partition_broadcast(l2a_b[:], l2a[:])
                nc.gpsimd.partition_broadcast(l2b_b[:], l2b[:])

                hn_sb = hnp.tile([128, FT, RC], BF, tag="hn")
                for f in range(FT):
                    nc.vector.tensor_tensor(hn_sb[:, f, :], h_sb[:, f, :],
                                            l2a_b[:], ALU.mult)
                    nc.vector.tensor_add(hn_sb[:, f, :], hn_sb[:, f, :],
                                         l2b_b[:])

                a_sb = ap_.tile([128, MT, RC], BF, tag="a")
                for m in range(MT):
                    ps = f1ps.tile([128, RC], F32, tag="f1")
                    for f in range(FT):
                        nc.tensor.matmul(ps[:], w1_sb[:, m, f, :],
                                         hn_sb[:, f, :],
                                         start=(f == 0), stop=(f == FT - 1))
                    nc.scalar.activation(a_sb[:, m, :], ps[:], AF.Relu,
                                         bias=bias_sb[:, m:m + 1])

                for dd in range(FT):
                    w2d = w2p.tile([128, MT, 128], BF, tag="w2d")
                    nc.sync.dma_start(out=w2d[:], in_=d["w2"].ap()[dd])
                    ps = f2ps.tile([128, RC], F32, tag="f2")
                    for t in range(MT):
                        nc.tensor.matmul(ps[:], w2d[:, t, :], a_sb[:, t, :],
                                         start=(t == 0), stop=(t == MT - 1))
                    o_t = outsp.tile([128, RC], F32, tag="ot")
                    nc.vector.scalar_tensor_tensor(
                        o_t[:], ps[:],
                        bias_sb[:, MT + dd:MT + dd + 1],
                        h_sb[:, dd, :], op0=ALU.add, op1=ALU.add)
                    nc.sync.dma_start(out=d["out"].ap()[:, dd], in_=o_t[:])


# ----------------------------------------------------------------------
# host side
# ----------------------------------------------------------------------

def make_in_maps(x, mask, Wq, Wk, Wv, Wo, w1, b1, w2, b2, g1, be1, g2, be2):
    """Build the 8 per-core input maps from the full inputs."""
    f32 = np.float32
    x = np.asarray(x, f32)
    mask = np.asarray(mask)
    Wq, Wk, Wv, Wo = (np.asarray(t, f32) for t in (Wq, Wk, Wv, Wo))
    w1, b1, w2, b2 = (np.asarray(t, f32) for t in (w1, b1, w2, b2))
    g1, be1, g2, be2 = (np.asarray(t, f32) for t in (g1, be1, g2, be2))

    Wq_s = g1[:, None] * Wq / np.sqrt(np.float32(DH))
    Wk_s = g1[:, None] * Wk
    Wv_s = g1[:, None] * Wv
    bq_full = (be1 @ Wq) / np.sqrt(np.float32(DH))
    bk_full = be1 @ Wk
    bv_full = be1 @ Wv
    w1_s = g2[:, None] * w1
    b1_s = b1 + be2 @ w1
    m2d = np.asarray(mask[0, 0], bool)
    mask4 = np.stack([m2d[0:512, 128 * j:128 * j + 128].T.astype(f32)
                      for j in range(4)]).astype(NPBF)
    recipd = np.full((128, 1), 1.0 / D, NPBF)
    ones64 = np.ones((128, 16, HC, 1), NPBF)
    b1t = np.ascontiguousarray(b1_s.reshape(MT, 128).T)
    b2t = np.ascontiguousarray(b2.reshape(FT, 128).T)
    # wo[p, j, g, dd, c] = Wo[(2g+j)*128+p, dd*128+c]
    wo_p = np.ascontiguousarray(
        Wo.reshape(TP, 2, 128, FT, 128).transpose(2, 1, 0, 3, 4)).astype(NPBF)
    w1_p = np.ascontiguousarray(
        w1_s.reshape(FT, 128, MT, 128).transpose(2, 1, 0, 3)).astype(NPBF)
    w2_p = np.ascontiguousarray(
        w2.reshape(MT, 128, FT, 128).transpose(2, 1, 0, 3)).astype(NPBF)

    in_maps = []
    for c in range(N_CORES):
        g, r = divmod(c, TP)
        xT = np.ascontiguousarray(x[g].T)                       # [D, S]
        xt = np.ascontiguousarray(
            xT.reshape(FT, 128, S).transpose(1, 0, 2)).astype(NPBF)
        xres = np.ascontiguousarray(
            xT[:, RC * r:RC * (r + 1)].reshape(FT, 128, RC)
            .transpose(1, 0, 2)).astype(NPBF)
        sh = slice(DC * r, DC * (r + 1))
        wq_c = np.ascontiguousarray(
            Wq_s[:, sh].reshape(FT, 128, 2, 128).transpose(2, 1, 0, 3)
        ).astype(NPBF)
        wk_c = np.ascontiguousarray(
            Wk_s[:, sh].reshape(FT, 128, 2, 128).transpose(2, 1, 0, 3)
        ).astype(NPBF)
        wv_c = np.ascontiguousarray(
            Wv_s[:, sh].reshape(FT, 128, DC).transpose(1, 0, 2)).astype(NPBF)
        qkvc = np.stack([Wq_s[:, sh].sum(0), Wk_s[:, sh].sum(0),
                         Wv_s[:, sh].sum(0), bq_full[sh], bk_full[sh],
                         bv_full[sh]]).astype(NPBF)
        in_maps.append({
            "xt": xt, "xres": xres, "wq": wq_c, "wk": wk_c, "wv": wv_c,
            "qkvc": qkvc, "wo": wo_p, "w1": w1_p, "b1t": b1t, "w2": w2_p,
            "b2t": b2t, "mask4": mask4, "recipd": recipd,
            "ones64": ones64,
            "colsel": np.array([[RC * r]], np.uint32),
        })
    return in_maps


def assemble_output(results):
    """[8 x {out: [128, FT, RC]}] -> [B, S, D] float32."""
    out = np.empty((B, S, D), np.float32)
    for c in range(N_CORES):
        g, r = divmod(c, TP)
        ot = results[c]["out"].transpose(1, 0, 2).reshape(D, RC)  # [D, RC]
        out[g, RC * r:RC * (r + 1), :] = ot.T
    return out


_nc_cache = {}


def get_nc(repeat=1, qkv_bias=False, **_ignored):
    key = (repeat, qkv_bias)
    if key not in _nc_cache:
        _nc_cache[key] = build(repeat=repeat, qkv_bias=qkv_bias)
    return _nc_cache[key]


def kernel(**inputs):
    qkv_bias = bool(np.any(np.asarray(inputs["be1"], np.float32)))
    nc = get_nc(qkv_bias=qkv_bias)
    in_maps = make_in_maps(**inputs)
    res = run_bass_kernel_spmd(nc, in_maps, core_ids=list(range(N_CORES)))
    return assemble_output(res.results)
